# revision 2
# baseline (speedup 1.0000x reference)
"""BlockTucker kernel for TRN2, 8 NeuronCores, data-parallel over batch.

Model (per reference):
    h0 = (x0 @ W0.T + b0).reshape(B, C, S)          B=8192 DIN=2048 MM=1600
    h1 = (x1 @ W1.T + b1).reshape(B, C, S)          C=20 chunks, S=80
    z[b,c,q] = sum_{s,t} h0[b,c,s] Wb[c,q,s,t] h1[b,c,t] + bb[c,q]
    z = signed_sqrt(z); z = z / max(||z||_chunk, eps); out = z @ Wout.T + bout

Per-core dataflow (BL = 1024 rows/core, all params replicated):
  stage A (PE, fp32r): h0T, h1T = [MM, BL] in DRAM (x/W transposed on-chip via
      PE-identity transposes; bias folded into the PSUM->SBUF eviction).
  middle, per chunk c / batch-tile bt:
      C1 (PE, fp32r): Y2[b, (q,t)] = h0T_c[s, b].T @ Wb[c,:,s,:]  (K=s=80, N=512)
      ACT evacuates PSUM -> SBUF casting to bf16.
      DVE: gating  G = Y2 * h1[b, t]  (bf16 2x mode, free-dim broadcast over q)
      DVE: log-tree add over t (80->40->20->10) + final reduce -> z[b, q] fp32
  tail (ACT/DVE): +bb, signed-sqrt, per-chunk L2 normalize  -> zn in DRAM
  out-proj (PE, fp32r): out = zn @ Wout.T + bout  (zn/Wout transposed on-chip)
"""

import numpy as np

BL = 1024          # batch rows per core
DIN = 2048
MM = 1600
C, S = 20, 80
OUT = 3000
NCORES = 8
EPS = 1e-12

_CACHE = {}


def _build():
    import concourse.bass as bass
    import concourse.mybir as mybir
    import concourse.tile as tile
    from concourse.masks import make_identity

    f32 = mybir.dt.float32
    f32r = mybir.dt.float32r
    bf16 = mybir.dt.bfloat16
    AF = mybir.ActivationFunctionType
    ALU = mybir.AluOpType
    AX = mybir.AxisListType

    nc = bass.Bass()

    x0 = nc.declare_dram_parameter("x0", [BL, DIN], f32, isOutput=False)
    x1 = nc.declare_dram_parameter("x1", [BL, DIN], f32, isOutput=False)
    W0 = nc.declare_dram_parameter("W0", [MM, DIN], f32, isOutput=False)
    b0 = nc.declare_dram_parameter("b0", [MM], f32, isOutput=False)
    W1 = nc.declare_dram_parameter("W1", [MM, DIN], f32, isOutput=False)
    b1 = nc.declare_dram_parameter("b1", [MM], f32, isOutput=False)
    Wb = nc.declare_dram_parameter("Wb", [C, S, S, S], f32r, isOutput=False)
    bb = nc.declare_dram_parameter("bb", [C, S], f32, isOutput=False)
    Wout = nc.declare_dram_parameter("Wout", [OUT, MM], f32, isOutput=False)
    bout = nc.declare_dram_parameter("bout", [OUT], f32, isOutput=False)
    out = nc.declare_dram_parameter("out", [BL, OUT], f32, isOutput=True)

    h0T = nc.dram_tensor("h0T", [MM, BL], f32r)
    h1T = nc.dram_tensor("h1T", [MM, BL], f32r)
    zb_d = nc.dram_tensor("zb_d", [BL, MM], f32)
    zn_d = nc.dram_tensor("zn_d", [BL, MM], f32)

    NKT = DIN // 128           # 16 k-tiles over DIN
    NMT = 13                   # m-tiles over MM: 12x128 + 64
    NBT = BL // 128            # 8 batch tiles
    QT = S * S                 # 6400 free (q,t) per chunk
    NJ = 13                    # (q,t) slices: 12x512 + 256
    NOG = 6                    # out slices: 5x512 + 440
    NOK = 13                   # k-tiles over MM for out-proj

    def msz(mt):
        return 128 if mt < NMT - 1 else MM - 128 * (NMT - 1)  # 64 tail

    def jsz(j):
        return 512 if j < NJ - 1 else QT - 512 * (NJ - 1)  # 256 tail

    def osz(og):
        return 512 if og < NOG - 1 else OUT - 512 * (NOG - 1)  # 440 tail

    with tile.TileContext(nc) as tc:
        from contextlib import ExitStack

        with ExitStack() as top:
            # ---- shared pools (live whole kernel) ----
            const = top.enter_context(tc.tile_pool(name="const", bufs=1))
            ps_t = top.enter_context(tc.tile_pool(name="ps_t", bufs=4, space="PSUM"))
            ps_mm = top.enter_context(tc.tile_pool(name="ps_mm", bufs=2, space="PSUM"))

            ident = const.tile([128, 128], f32)
            make_identity(nc, ident)
            identR = const.tile([128, 128], f32r)
            nc.scalar.copy(identR[:], ident[:])

            # biases for stage A: [128, 13] layout, col j holds m = j*128 + p
            b0sb = const.tile([128, NMT], f32)
            b1sb = const.tile([128, NMT], f32)
            for bsrc, bdst in ((b0, b0sb), (b1, b1sb)):
                nc.sync.dma_start(
                    out=bdst[:, : NMT - 1],
                    in_=bsrc[: 128 * (NMT - 1)].rearrange("(j p) -> p j", p=128),
                )
                nc.sync.dma_start(
                    out=bdst[: msz(NMT - 1), NMT - 1 : NMT],
                    in_=bsrc[128 * (NMT - 1) :].unsqueeze(1),
                )
            # bb replicated across partitions: [128, 1600] (c,q) flattened
            bbrep = const.tile([128, MM], f32)
            nc.sync.dma_start(
                out=bbrep[:],
                in_=bb[:].rearrange("c q -> (c q)").unsqueeze(0).broadcast_to([128, MM]),
            )
            # bout replicated: [128, 3000]
            borep = const.tile([128, OUT], f32)
            nc.sync.dma_start(
                out=borep[:],
                in_=bout[:].unsqueeze(0).broadcast_to([128, OUT]),
            )

            # ================= stage A: hT = (x @ W.T + b).T =================
            def stage_a(x_d, W_d, bias_sb, hT_d, tag):
                with ExitStack() as ctx:
                    big = ctx.enter_context(tc.tile_pool(name=f"stA_xT{tag}", bufs=1))
                    ld = ctx.enter_context(tc.tile_pool(name=f"stA_ld{tag}", bufs=4))
                    wld = ctx.enter_context(tc.tile_pool(name=f"stA_wld{tag}", bufs=4))
                    wtp = ctx.enter_context(tc.tile_pool(name=f"stA_wt{tag}", bufs=5))
                    ev = ctx.enter_context(tc.tile_pool(name=f"stA_ev{tag}", bufs=4))

                    xT = big.tile([128, NKT, BL], f32r)  # 64KB/part
                    for bt in range(NBT):
                        xn = ld.tile([128, DIN], f32, tag="xn")
                        nc.sync.dma_start(
                            out=xn[:], in_=x_d[bt * 128 : (bt + 1) * 128, :]
                        )
                        for k4 in range(NKT // 4):
                            pst = ps_t.tile([128, 512], f32, tag="tp")
                            for h in range(4):
                                k = 4 * k4 + h
                                nc.tensor.transpose(
                                    pst[:, h * 128 : (h + 1) * 128],
                                    xn[:, k * 128 : (k + 1) * 128],
                                    ident[:],
                                )
                            nc.scalar.copy(
                                xT[:, 4 * k4 : 4 * k4 + 4, bt * 128 : (bt + 1) * 128],
                                pst[:].rearrange("p (a b) -> p a b", a=4),
                            )
                    for mt in range(NMT):
                        ms = msz(mt)
                        wn = wld.tile([128, DIN], f32, tag="wn")
                        nc.sync.dma_start(
                            out=wn[:ms, :],
                            in_=W_d[mt * 128 : mt * 128 + ms, :],
                        )
                        ps01 = ps_mm.tile([128, 1024], f32, tag="mm")
                        for k4 in range(NKT // 4):
                            pst = ps_t.tile([128, 512], f32, tag="tp")
                            for h in range(4):
                                k = 4 * k4 + h
                                nc.tensor.transpose(
                                    pst[:, h * 128 : h * 128 + ms],
                                    wn[:ms, k * 128 : (k + 1) * 128],
                                    ident[:ms, :ms],
                                )
                            wt = wtp.tile([128, 512], f32r, tag="wt")
                            nc.scalar.copy(wt[:], pst[:])
                            for h in range(4):
                                k = 4 * k4 + h
                                nc.tensor.matmul(
                                    ps01[:ms, :512],
                                    lhsT=wt[:, h * 128 : h * 128 + ms],
                                    rhs=xT[:, k, :512],
                                    start=(k == 0),
                                    stop=(k == NKT - 1),
                                )
                                nc.tensor.matmul(
                                    ps01[:ms, 512:],
                                    lhsT=wt[:, h * 128 : h * 128 + ms],
                                    rhs=xT[:, k, 512:],
                                    start=(k == 0),
                                    stop=(k == NKT - 1),
                                )
                        evt = ev.tile([128, BL], f32r, tag="ev")
                        nc.scalar.activation(
                            evt[:ms, :], ps01[:ms, :], AF.Identity,
                            bias=bias_sb[:ms, mt : mt + 1],
                        )
                        nc.sync.dma_start(
                            out=hT_d[mt * 128 : mt * 128 + ms, :], in_=evt[:ms, :]
                        )

            with ExitStack() as actx:
                stage_a(x0, W0, b0sb, h0T, 0)
                stage_a(x1, W1, b1sb, h1T, 1)

            # ================= middle: bilinear per chunk =================
            with ExitStack() as ctx:
                wbsp = ctx.enter_context(tc.tile_pool(name="wbs", bufs=2))
                h0p = ctx.enter_context(tc.tile_pool(name="h0c", bufs=3))
                h1p = ctx.enter_context(tc.tile_pool(name="h1n", bufs=3))
                h1bp = ctx.enter_context(tc.tile_pool(name="h1b", bufs=3))
                y2p = ctx.enter_context(tc.tile_pool(name="y2", bufs=2))
                gp = ctx.enter_context(tc.tile_pool(name="g", bufs=2))
                t1p = ctx.enter_context(tc.tile_pool(name="t1", bufs=2))
                t2p = ctx.enter_context(tc.tile_pool(name="t2", bufs=2))
                t3p = ctx.enter_context(tc.tile_pool(name="t3", bufs=2))
                zsp = ctx.enter_context(tc.tile_pool(name="zst", bufs=3))

                for c in range(C):
                    wbs = wbsp.tile([S, S, S], f32r, tag="wbs")  # [s, q, t]
                    nc.sync.dma_start(
                        out=wbs[:], in_=Wb[c].rearrange("q s t -> s q t")
                    )
                    wbs_f = wbs[:].rearrange("s q t -> s (q t)")
                    csl = slice(c * S, (c + 1) * S)
                    h0cw = h0p.tile([S, BL], f32r, tag="h0c")
                    nc.sync.dma_start(out=h0cw[:], in_=h0T[csl, :])
                    h1nw = h1p.tile([S, BL], f32r, tag="h1n")
                    nc.sync.dma_start(out=h1nw[:], in_=h1T[csl, :])
                    for bt in range(NBT):
                        bsl = slice(bt * 128, (bt + 1) * 128)
                        pst = ps_t.tile([128, 128], f32r, tag="tp")
                        nc.tensor.transpose(
                            pst[:, :S], h1nw[:, bsl], identR[:S, :S]
                        )
                        h1b = h1bp.tile([128, S], bf16, tag="h1b")
                        nc.scalar.copy(h1b[:], pst[:, :S])

                        g = gp.tile([128, S, S], bf16, tag="g")
                        y2 = y2p.tile([128, QT], bf16, tag="y2")
                        for j2 in range(7):
                            w2 = 1024 if j2 < 6 else 256
                            ps = ps_mm.tile([128, 1024], f32, tag="mm")
                            for h in range(2):
                                j = 2 * j2 + h
                                if j >= NJ:
                                    continue
                                w = jsz(j)
                                nc.tensor.matmul(
                                    ps[:, h * 512 : h * 512 + w],
                                    lhsT=h0cw[:, bsl],
                                    rhs=wbs_f[:, j * 512 : j * 512 + w],
                                    start=True,
                                    stop=True,
                                )
                            nc.scalar.copy(
                                y2[:, j2 * 1024 : j2 * 1024 + w2], ps[:, :w2]
                            )
                        y23 = y2[:].rearrange("p (q t) -> p q t", t=S)
                        nc.vector.tensor_tensor(
                            out=g[:],
                            in0=y23,
                            in1=h1b[:].unsqueeze(1).broadcast_to([128, S, S]),
                            op=ALU.mult,
                        )
                        t1 = t1p.tile([128, S, 40], bf16, tag="t1")
                        nc.vector.tensor_tensor(
                            out=t1[:], in0=g[:, :, :40], in1=g[:, :, 40:], op=ALU.add
                        )
                        t2 = t2p.tile([128, S, 20], bf16, tag="t2")
                        nc.vector.tensor_tensor(
                            out=t2[:], in0=t1[:, :, :20], in1=t1[:, :, 20:], op=ALU.add
                        )
                        t3 = t3p.tile([128, S, 10], bf16, tag="t3")
                        nc.vector.tensor_tensor(
                            out=t3[:], in0=t2[:, :, :10], in1=t2[:, :, 10:], op=ALU.add
                        )
                        zst = zsp.tile([128, S], f32, tag="zst")
                        nc.vector.tensor_reduce(
                            out=zst[:], in_=t3[:], axis=AX.X, op=ALU.add
                        )
                        nc.sync.dma_start(out=zb_d[bsl, csl], in_=zst[:])

            # ===== tail (+bb, signed sqrt, normalize) fused with out-proj ====
            with ExitStack() as ctx:
                zp = ctx.enter_context(tc.tile_pool(name="tail", bufs=2))
                sp = ctx.enter_context(tc.tile_pool(name="tails", bufs=2))

                for bt in range(NBT):
                    bsl = slice(bt * 128, (bt + 1) * 128)
                    zt = zp.tile([128, MM], f32, tag="zt")
                    nc.sync.dma_start(out=zt[:], in_=zb_d[bsl, :])
                    zbb = zp.tile([128, MM], f32, tag="zbb")
                    nc.vector.tensor_tensor(
                        out=zbb[:], in0=zt[:], in1=bbrep[:], op=ALU.add
                    )
                    sgn = zp.tile([128, MM], f32, tag="sgn")
                    nc.scalar.activation(sgn[:], zbb[:], AF.Sign)
                    sab = zp.tile([128, MM], f32, tag="zt")
                    nc.scalar.activation(sab[:], zbb[:], AF.Abs)
                    ssq = zp.tile([128, MM], f32, tag="zbb")
                    nc.scalar.activation(ssq[:], sab[:], AF.Sqrt)
                    ss = zp.tile([128, MM], f32, tag="sgn")
                    nc.vector.tensor_tensor(
                        out=ss[:], in0=sgn[:], in1=ssq[:], op=ALU.mult
                    )
                    # ||ss||^2 per chunk = sum |zbb| per chunk
                    nsq = sp.tile([128, C], f32, tag="nsq")
                    nc.vector.tensor_reduce(
                        out=nsq[:],
                        in_=zbb[:].rearrange("p (c q) -> p c q", q=S),
                        axis=AX.X,
                        op=ALU.add,
                        apply_absolute_value=True,
                    )
                    nrm = sp.tile([128, C], f32, tag="nrm")
                    nc.scalar.activation(nrm[:], nsq[:], AF.Sqrt)
                    nrmc = sp.tile([128, C], f32, tag="nrmc")
                    nc.vector.tensor_scalar_max(out=nrmc[:], in0=nrm[:], scalar1=EPS)
                    inv = sp.tile([128, C], f32, tag="inv")
                    nc.vector.reciprocal(inv[:], nrmc[:])
                    zn = zp.tile([128, MM], f32, tag="zn")
                    nc.vector.tensor_tensor(
                        out=zn[:].rearrange("p (c q) -> p c q", q=S),
                        in0=ss[:].rearrange("p (c q) -> p c q", q=S),
                        in1=inv[:].unsqueeze(2).broadcast_to([128, C, S]),
                        op=ALU.mult,
                    )
                    nc.sync.dma_start(out=zn_d[bsl, :], in_=zn[:])

                # ---- out-proj: out = zn @ Wout.T + bout (same pool scope
                # as the tail so the two phases can overlap) ----
                big = ctx.enter_context(tc.tile_pool(name="znT", bufs=1))
                ld = ctx.enter_context(tc.tile_pool(name="op_ld", bufs=2))
                wnp = ctx.enter_context(tc.tile_pool(name="op_wn", bufs=1))
                wop = ctx.enter_context(tc.tile_pool(name="op_w", bufs=1))
                evp = ctx.enter_context(tc.tile_pool(name="op_ev", bufs=3))

                znT = big.tile([128, NOK, BL], f32r)  # 52KB/part
                for bt in range(NBT):
                    znn = ld.tile([128, MM], f32, tag="znn")
                    nc.sync.dma_start(
                        out=znn[:], in_=zn_d[bt * 128 : (bt + 1) * 128, :]
                    )
                    for k in range(NOK):
                        ks = msz(k)
                        pst = ps_t.tile([128, 128], f32, tag="tp")
                        nc.tensor.transpose(
                            pst[:ks, :],
                            znn[:, k * 128 : k * 128 + ks],
                            ident[:],
                        )
                        nc.scalar.copy(
                            znT[:ks, k, bt * 128 : (bt + 1) * 128], pst[:ks, :]
                        )

                for og in range(NOG):
                    ow = osz(og)
                    not_ = (ow + 127) // 128
                    wnb = wnp.tile([128, 4, MM], f32, tag="wno")  # og rows x all k
                    for ot in range(not_):
                        os_ = min(128, ow - ot * 128)
                        nc.sync.dma_start(
                            out=wnb[:os_, ot, :],
                            in_=Wout[
                                og * 512 + ot * 128 : og * 512 + ot * 128 + os_, :
                            ],
                        )
                    woT = wop.tile([128, NOK, 512], f32r, tag="woT")
                    for k in range(NOK):
                        ks = msz(k)
                        pst = ps_t.tile([128, 512], f32, tag="tp")
                        for ot in range(not_):
                            os_ = min(128, ow - ot * 128)
                            nc.tensor.transpose(
                                pst[:ks, ot * 128 : ot * 128 + os_],
                                wnb[:os_, ot, k * 128 : k * 128 + ks],
                                ident[:os_, :os_],
                            )
                        nc.scalar.copy(woT[:ks, k, :ow], pst[:ks, :ow])
                    for bt in range(NBT):
                        ps = ps_mm.tile([128, 1024], f32, tag="mm")
                        for k in range(NOK):
                            ks = msz(k)
                            nc.tensor.matmul(
                                ps[:, :ow],
                                lhsT=znT[:ks, k, bt * 128 : (bt + 1) * 128],
                                rhs=woT[:ks, k, :ow],
                                start=(k == 0),
                                stop=(k == NOK - 1),
                            )
                        evt = evp.tile([128, 512], f32, tag="evo")
                        nc.vector.tensor_tensor(
                            out=evt[:, :ow],
                            in0=ps[:, :ow],
                            in1=borep[:, og * 512 : og * 512 + ow],
                            op=ALU.add,
                        )
                        nc.sync.dma_start(
                            out=out[
                                bt * 128 : (bt + 1) * 128,
                                og * 512 : og * 512 + ow,
                            ],
                            in_=evt[:, :ow],
                        )

    _split_excess_waits(nc, cap=4)
    return nc


def _split_excess_waits(nc, cap=4):
    """Walrus rejects instructions with too many sync waits. Move excess
    waits onto NoOps spliced just before the instruction on the same engine
    queue (the sequencer executes them in order, so semantics are identical).
    """
    import concourse.mybir as mybir
    import bass_rust

    n = 0
    for f in nc.m.functions:
        for blk in f.blocks:
            out = []
            changed = False
            for inst in blk.instructions:
                si = getattr(inst, "sync_info", None)
                waits = list(si.on_wait) if si is not None and si.on_wait else []
                icap = 2 if inst.opcode == "EventSemaphore" else 1
                if len(waits) > icap:
                    excess, keep = waits[:-icap], waits[-icap:]
                    for w in excess:
                        nop = mybir.InstNoOp(
                            name=f"{inst.name}-wsplit{n}", ins=[], outs=[]
                        )
                        n += 1
                        nop.engine = inst.engine
                        nop.sync_info = bass_rust.SyncInfo(
                            on_wait=[w], on_update=[]
                        )
                        out.append(nop)
                    inst.sync_info = bass_rust.SyncInfo(
                        on_wait=keep, on_update=list(si.on_update or [])
                    )
                    changed = True
                out.append(inst)
            if changed:
                blk.instructions = out
    return nc


def _get_nc():
    if "nc" not in _CACHE:
        _CACHE["nc"] = _build()
    return _CACHE["nc"]


def _shard_inputs(inputs):
    full = {k: np.ascontiguousarray(np.asarray(v, dtype=np.float32)) for k, v in inputs.items()}
    rows = full["x0"].shape[0] // NCORES
    in_maps = []
    for i in range(NCORES):
        m = dict(full)
        m["x0"] = np.ascontiguousarray(full["x0"][i * rows : (i + 1) * rows])
        m["x1"] = np.ascontiguousarray(full["x1"][i * rows : (i + 1) * rows])
        in_maps.append(m)
    return in_maps


def kernel(**inputs):
    from concourse.bass_utils import run_bass_kernel_spmd

    nc = _get_nc()
    in_maps = _shard_inputs(inputs)
    res = run_bass_kernel_spmd(nc, in_maps, list(range(NCORES)))
    return np.concatenate([res.results[i]["out"] for i in range(NCORES)], axis=0)

